# revision 15
# baseline (speedup 1.0000x reference)
"""MoE layer (top-1 routing) on 8 Trainium2 NeuronCores.

Strategy (expert parallelism): the router runs on host (cheap:
16384x1024x8 matmul) to obtain each token's expert and gate weight.
Tokens are packed per-expert and dispatched so core e holds expert e's
tokens (zero-padded to a common capacity C). Each core then runs its
expert's FFN over its tokens in one fused program:

    pass 1:  hT = relu(W1.T @ xT + b1)      [F, C] fp16, spilled to DRAM
    pass 2:  yT = (W2.T @ hT + b2) * w      [D, C] fp32

Activations stay feature-major ([feature, token]) so both matmuls use
natural weight layouts as the stationary operand and token-tiles as the
moving operand. Matmul operands are fp16 (full PE rate, fp32 PSUM
accumulation; measured end-to-end rel err ~2.5e-4). W2 stays resident
in SBUF and is prefetched during pass 1. The host scatters per-core
outputs back to the original token order.
"""
import numpy as np

import concourse.bacc as bacc
import concourse.mybir as mybir
import concourse.tile as tile
from concourse import bass_utils

N_CORES = 8
T, D, F, E = 16384, 1024, 4096, 8
F32 = mybir.dt.float32
F32R = mybir.dt.float32r
F16 = mybir.dt.float16
RELU = mybir.ActivationFunctionType.Relu

# "fp16": matmul operands in float16   (1.0 PE cycles/row, rel err ~2.5e-4)
# "f32r": matmul operands in float32r  (~1.3 PE cycles/row, rel err ~1.3e-4)
MM_MODE = "fp16"

_PROGS: dict = {}


def _mm_dt():
    return F16 if MM_MODE == "fp16" else F32R


def _np_mm(a):
    return np.asarray(a, np.float16 if MM_MODE == "fp16" else np.float32)


def _groups(C, step=512):
    out, c = [], 0
    while c < C:
        out.append((c, min(step, C - c)))
        c += step
    return out


def build(C, loop_n=1):
    nc = bacc.Bacc("TRN2", target_bir_lowering=False, debug=False,
                   num_devices=N_CORES)
    MDT = _mm_dt()
    xT = nc.dram_tensor("xT", [D, C], MDT, kind="ExternalInput").ap()
    w1h = nc.dram_tensor("w1h", [F, D], MDT, kind="ExternalInput").ap()
    w2 = nc.dram_tensor("w2", [F, D], MDT, kind="ExternalInput").ap()
    b1h = nc.dram_tensor("b1h", [128, F // 128], F32, kind="ExternalInput").ap()
    b2h = nc.dram_tensor("b2h", [128, D // 128], F32, kind="ExternalInput").ap()
    wb = nc.dram_tensor("wb", [128, C], F32, kind="ExternalInput").ap()
    yT = nc.dram_tensor("yT", [D, C], F32, kind="ExternalOutput").ap()
    gs = _groups(C)
    ng = len(gs)
    with tile.TileContext(nc) as tc:
        with (
            tc.tile_pool(name="const", bufs=1) as constp,
            tc.tile_pool(name="w2", bufs=1) as w2p,
            tc.tile_pool(name="x", bufs=1) as xp,
            tc.tile_pool(name="w1", bufs=3) as w1p,
            tc.tile_pool(name="hs", bufs=4) as hsp,
            tc.tile_pool(name="hkeep", bufs=1) as hkp,
            tc.tile_pool(name="hl", bufs=6) as hlp,
            tc.tile_pool(name="ys", bufs=4) as ysp,
            tc.tile_pool(name="ps", bufs=1, space="PSUM") as psp,
            tc.tile_pool(name="hdram", bufs=1, space="DRAM") as hdram,
        ):
            def body(_=None):
                hT = hdram.tile([F, C], MDT, tag="hT", name="hT")
                b1t = constp.tile([128, F // 128], F32, tag="b1", name="b1t")
                nc.sync.dma_start(b1t[:], b1h[:])
                b2t = constp.tile([128, D // 128], F32, tag="b2", name="b2t")
                nc.sync.dma_start(b2t[:], b2h[:])
                wbt = constp.tile([128, C], F32, tag="wb", name="wbt")
                nc.sync.dma_start(wbt[:], wb[:])
                xall = xp.tile([128, 8, C], MDT, tag="x", name="xall")
                w1q = {}
                w2ts = [None] * 32
                hkeep = [None] * 32

                def issue_w1(ft):
                    t = w1p.tile([128, D], MDT, tag="w1", name=f"w1_{ft}")
                    nc.sync.dma_start(t[:], w1h[ft * 128:(ft + 1) * 128, :])
                    w1q[ft] = t

                def issue_w2(ft):
                    t = w2p.tile([128, D], MDT, tag=f"w2_{ft}",
                                 name=f"w2t_{ft}")
                    nc.sync.dma_start(t[:], w2[ft * 128:(ft + 1) * 128, :])
                    w2ts[ft] = t

                # sync ring is FIFO: w1[0], group-0 x, w1[1], rest of x —
                # keeps the first matmul chains fed
                issue_w1(0)
                for (c0, cl) in gs[:1]:
                    for dt in range(8):
                        nc.sync.dma_start(
                            xall[:, dt, c0:c0 + cl],
                            xT[dt * 128:(dt + 1) * 128, c0:c0 + cl])
                issue_w1(1)
                for (c0, cl) in gs[1:]:
                    for dt in range(8):
                        nc.sync.dma_start(
                            xall[:, dt, c0:c0 + cl],
                            xT[dt * 128:(dt + 1) * 128, c0:c0 + cl])

                # ---- pass 1: hT = relu(W1.T @ xT + b1), W2 prefetch paced
                for ft in range(32):
                    w1t = w1q.pop(ft)
                    if ft + 2 < 32:
                        issue_w1(ft + 2)
                    issue_w2(ft)
                    for gi, (c0, cl) in enumerate(gs):
                        ps = psp.tile([128, 512], F32,
                                      tag=f"psy{(ft * ng + gi) % 8}",
                                      name=f"ps1_{ft}_{gi}")
                        for dt in range(8):
                            nc.tensor.matmul(
                                ps[:, :cl],
                                w1t[:, dt * 128:(dt + 1) * 128],
                                xall[:, dt, c0:c0 + cl],
                                start=(dt == 0), stop=(dt == 7))
                        if gi == 0:
                            # group 0 stays SBUF-resident for pass 2
                            hs = hkp.tile([128, 512], MDT, tag=f"hk_{ft}",
                                          name=f"hk_{ft}")
                            hkeep[ft] = hs
                        else:
                            hs = hsp.tile([128, 512], MDT, tag="hs",
                                          name=f"hs_{ft}_{gi}")
                        nc.scalar.activation(hs[:, :cl], ps[:, :cl], RELU,
                                             bias=b1t[:, ft:ft + 1])
                        if gi > 0:
                            nc.sync.dma_start(
                                hT[ft * 128:(ft + 1) * 128, c0:c0 + cl],
                                hs[:, :cl])

                # ---- pass 2: yT = (W2.T @ hT + b2) * w
                for gi, (c0, cl) in enumerate(gs):
                    psys = [psp.tile([128, 512], F32, tag=f"psy{dm}",
                                     name=f"ps2_{dm}_{gi}")
                            for dm in range(8)]
                    for ft in range(32):
                        if gi == 0:
                            hl = hkeep[ft]
                        else:
                            hl = hlp.tile([128, 512], MDT, tag="hl",
                                          name=f"hl_{ft}_{gi}")
                            nc.sync.dma_start(
                                hl[:, :cl],
                                hT[ft * 128:(ft + 1) * 128, c0:c0 + cl])
                        for dm in range(8):
                            nc.tensor.matmul(
                                psys[dm][:, :cl],
                                w2ts[ft][:, dm * 128:(dm + 1) * 128],
                                hl[:, :cl],
                                start=(ft == 0), stop=(ft == 31))
                    for dm in range(8):
                        ys = ysp.tile([128, 512], F32, tag="ys",
                                      name=f"ys_{dm}_{gi}")
                        nc.vector.tensor_scalar_add(ys[:, :cl],
                                                    psys[dm][:, :cl],
                                                    b2t[:, dm:dm + 1])
                        nc.vector.tensor_mul(ys[:, :cl], ys[:, :cl],
                                             wbt[:, c0:c0 + cl])
                        nc.sync.dma_start(
                            yT[dm * 128:(dm + 1) * 128, c0:c0 + cl],
                            ys[:, :cl])

            if loop_n == 1:
                body()
            else:
                with tc.For_i(0, loop_n, 1):
                    body()
    nc.compile()
    return nc


def get_progs(C, loop_n=1):
    key = (C, loop_n, MM_MODE)
    if key not in _PROGS:
        _PROGS[key] = build(C, loop_n)
    return _PROGS[key]


def route(x, Wr, br):
    """Host router: top-1 expert index and gate weight (mirrors reference)."""
    logits = x.astype(np.float32) @ Wr.astype(np.float32) + br.astype(np.float32)
    m = logits.max(axis=-1, keepdims=True)
    ex = np.exp(logits - m)
    p = ex / ex.sum(axis=-1, keepdims=True)
    idx = np.argmax(p, axis=-1)
    w = p[np.arange(p.shape[0]), idx]
    return idx, w


# SBUF budget caps the per-core token capacity of one program invocation;
# pathological routing (an expert drawing >C_MAX tokens) falls back to
# running the same program over multiple token chunks.
C_MAX = 4096


def _pack(x, w, sels, C, W1, b1, W2, b2):
    ins = []
    for e in range(E):
        sel = sels[e]
        n = len(sel)
        xTe = np.zeros((D, C), np.float16 if MM_MODE == "fp16" else np.float32)
        xTe[:, :n] = _np_mm(x[sel].T)
        w1h = np.ascontiguousarray(
            _np_mm(W1[e]).reshape(8, 128, 32, 128)
            .transpose(2, 1, 0, 3).reshape(F, D))
        b1h = np.ascontiguousarray(
            np.asarray(b1[e], np.float32).reshape(32, 128).T)
        b2h = np.ascontiguousarray(
            np.asarray(b2[e], np.float32).reshape(8, 128).T)
        wbe = np.zeros((128, C), np.float32)
        wbe[:, :n] = np.asarray(w[sel], np.float32)[None, :]
        ins.append({"xT": xTe, "w1h": w1h,
                    "w2": np.ascontiguousarray(_np_mm(W2[e])),
                    "b1h": b1h, "b2h": b2h, "wb": wbe})
    return ins


def prepare(x, Wr, br, W1, b1, W2, b2):
    """Host routing + per-core input packing (single-chunk case)."""
    x = np.ascontiguousarray(np.asarray(x, np.float32))
    idx, w = route(x, np.asarray(Wr, np.float32), np.asarray(br, np.float32))
    perm = np.argsort(idx, kind="stable")
    counts = np.bincount(idx, minlength=E)
    C = int(max(256, -(-counts.max() // 256) * 256))
    offs = np.zeros(E + 1, np.int64)
    offs[1:] = np.cumsum(counts)
    sels = [perm[offs[e]:offs[e + 1]] for e in range(E)]
    ins = _pack(x, w, sels, min(C, C_MAX), W1, b1, W2, b2)
    return ins, perm, counts, C


def kernel(x, Wr, br, W1, b1, W2, b2):
    x = np.ascontiguousarray(np.asarray(x, np.float32))
    idx, w = route(x, np.asarray(Wr, np.float32), np.asarray(br, np.float32))
    perm = np.argsort(idx, kind="stable")
    counts = np.bincount(idx, minlength=E)
    offs = np.zeros(E + 1, np.int64)
    offs[1:] = np.cumsum(counts)
    sels = [perm[offs[e]:offs[e + 1]] for e in range(E)]
    out = np.zeros((x.shape[0], D), np.float32)
    n_chunks = max(1, -(-int(counts.max()) // C_MAX))
    for ci in range(n_chunks):
        cs = [s[ci * C_MAX:(ci + 1) * C_MAX] for s in sels]
        cmax = max(len(s) for s in cs)
        C = int(max(256, -(-cmax // 256) * 256))
        ins = _pack(x, w, cs, C, W1, b1, W2, b2)
        nc = get_progs(C)
        r = bass_utils.run_bass_kernel_spmd(nc, ins, list(range(N_CORES)))
        for e in range(E):
            n = len(cs[e])
            if n:
                out[cs[e]] = r.results[e]["yT"][:, :n].T
    return out


# revision 16
# speedup vs baseline: 1.2975x; 1.2975x over previous
"""MoE layer (top-1 routing) on 8 Trainium2 NeuronCores.

Strategy (expert parallelism): the router runs on host (cheap:
16384x1024x8 matmul) to obtain each token's expert and gate weight.
Tokens are packed per-expert and dispatched so core e holds expert e's
tokens (zero-padded to a common capacity C). Each core then runs its
expert's FFN over its tokens in one fused program:

    pass 1:  hT = relu(W1.T @ xT + b1)      [F, C] fp16, spilled to DRAM
    pass 2:  yT = (W2.T @ hT + b2) * w      [D, C] fp32

Activations stay feature-major ([feature, token]) so both matmuls use
natural weight layouts as the stationary operand and token-tiles as the
moving operand. Matmul operands are fp16 (full PE rate, fp32 PSUM
accumulation; measured end-to-end rel err ~2.5e-4). W2 stays resident
in SBUF and is prefetched during pass 1. The host scatters per-core
outputs back to the original token order.
"""
import numpy as np

import concourse.bacc as bacc
import concourse.mybir as mybir
import concourse.tile as tile
from concourse import bass_utils

N_CORES = 8
T, D, F, E = 16384, 1024, 4096, 8
F32 = mybir.dt.float32
F32R = mybir.dt.float32r
F16 = mybir.dt.float16
RELU = mybir.ActivationFunctionType.Relu

# "fp16": matmul operands in float16   (1.0 PE cycles/row, rel err ~2.5e-4)
# "f32r": matmul operands in float32r  (~1.3 PE cycles/row, rel err ~1.3e-4)
MM_MODE = "fp16"

_PROGS: dict = {}


def _mm_dt():
    return F16 if MM_MODE == "fp16" else F32R


def _np_mm(a):
    return np.asarray(a, np.float16 if MM_MODE == "fp16" else np.float32)


def _groups(C, step=512):
    out, c = [], 0
    while c < C:
        out.append((c, min(step, C - c)))
        c += step
    return out


def build(C, loop_n=1):
    nc = bacc.Bacc("TRN2", target_bir_lowering=False, debug=False,
                   num_devices=N_CORES)
    MDT = _mm_dt()
    xT = nc.dram_tensor("xT", [D, C], MDT, kind="ExternalInput").ap()
    w1h = nc.dram_tensor("w1h", [F, D], MDT, kind="ExternalInput").ap()
    w2 = nc.dram_tensor("w2", [F, D], MDT, kind="ExternalInput").ap()
    b1h = nc.dram_tensor("b1h", [128, F // 128], F32, kind="ExternalInput").ap()
    b2h = nc.dram_tensor("b2h", [128, D // 128], F32, kind="ExternalInput").ap()
    wb = nc.dram_tensor("wb", [128, C], F32, kind="ExternalInput").ap()
    yT = nc.dram_tensor("yT", [D, C], F32, kind="ExternalOutput").ap()
    gs = _groups(C)
    ng = len(gs)
    with tile.TileContext(nc) as tc:
        with (
            tc.tile_pool(name="const", bufs=1) as constp,
            tc.tile_pool(name="w2", bufs=1) as w2p,
            tc.tile_pool(name="x", bufs=1) as xp,
            tc.tile_pool(name="w1", bufs=4) as w1p,
            tc.tile_pool(name="hs", bufs=6) as hsp,
            tc.tile_pool(name="hkeep", bufs=1) as hkp,
            tc.tile_pool(name="hl", bufs=10) as hlp,
            tc.tile_pool(name="ys", bufs=4) as ysp,
            tc.tile_pool(name="ps", bufs=1, space="PSUM") as psp,
            tc.tile_pool(name="hdram", bufs=1, space="DRAM") as hdram,
        ):
            def body(_=None):
                hT = hdram.tile([F, C], MDT, tag="hT", name="hT")
                b1t = constp.tile([128, F // 128], F32, tag="b1", name="b1t")
                nc.sync.dma_start(b1t[:], b1h[:])
                b2t = constp.tile([128, D // 128], F32, tag="b2", name="b2t")
                nc.sync.dma_start(b2t[:], b2h[:])
                wbt = constp.tile([128, C], F32, tag="wb", name="wbt")
                nc.sync.dma_start(wbt[:], wb[:])
                xall = xp.tile([128, 8, C], MDT, tag="x", name="xall")
                w1q = {}
                w2ts = [None] * 32
                hkeep = [None] * 32

                def issue_w1(ft):
                    t = w1p.tile([128, D], MDT, tag="w1", name=f"w1_{ft}")
                    nc.sync.dma_start(t[:], w1h[ft * 128:(ft + 1) * 128, :])
                    w1q[ft] = t

                def issue_w2(ft):
                    t = w2p.tile([128, D], MDT, tag=f"w2_{ft}",
                                 name=f"w2t_{ft}")
                    nc.sync.dma_start(t[:], w2[ft * 128:(ft + 1) * 128, :])
                    w2ts[ft] = t

                # sync ring is FIFO: w1[0], group-0 x, w1[1], rest of x —
                # keeps the first matmul chains fed
                issue_w1(0)
                issue_w1(1)
                for (c0, cl) in gs[:1]:
                    for dt in range(8):
                        nc.sync.dma_start(
                            xall[:, dt, c0:c0 + cl],
                            xT[dt * 128:(dt + 1) * 128, c0:c0 + cl])
                issue_w1(2)
                for (c0, cl) in gs[1:]:
                    for dt in range(8):
                        nc.sync.dma_start(
                            xall[:, dt, c0:c0 + cl],
                            xT[dt * 128:(dt + 1) * 128, c0:c0 + cl])

                # ---- pass 1: hT = relu(W1.T @ xT + b1), W2 prefetch paced
                for ft in range(32):
                    w1t = w1q.pop(ft)
                    if ft + 3 < 32:
                        issue_w1(ft + 3)
                    issue_w2(ft)
                    for gi, (c0, cl) in enumerate(gs):
                        ps = psp.tile([128, 512], F32,
                                      tag=f"psy{(ft * ng + gi) % 8}",
                                      name=f"ps1_{ft}_{gi}")
                        for dt in range(8):
                            nc.tensor.matmul(
                                ps[:, :cl],
                                w1t[:, dt * 128:(dt + 1) * 128],
                                xall[:, dt, c0:c0 + cl],
                                start=(dt == 0), stop=(dt == 7))
                        if gi == 0:
                            # group 0 stays SBUF-resident for pass 2
                            hs = hkp.tile([128, 512], MDT, tag=f"hk_{ft}",
                                          name=f"hk_{ft}")
                            hkeep[ft] = hs
                        else:
                            hs = hsp.tile([128, 512], MDT, tag="hs",
                                          name=f"hs_{ft}_{gi}")
                        nc.scalar.activation(hs[:, :cl], ps[:, :cl], RELU,
                                             bias=b1t[:, ft:ft + 1])
                        if gi > 0:
                            nc.sync.dma_start(
                                hT[ft * 128:(ft + 1) * 128, c0:c0 + cl],
                                hs[:, :cl])

                # ---- pass 2: yT = (W2.T @ hT + b2) * w
                for gi, (c0, cl) in enumerate(gs):
                    psys = [psp.tile([128, 512], F32, tag=f"psy{dm}",
                                     name=f"ps2_{dm}_{gi}")
                            for dm in range(8)]
                    for ft in range(32):
                        if gi == 0:
                            hl = hkeep[ft]
                        else:
                            hl = hlp.tile([128, 512], MDT, tag="hl",
                                          name=f"hl_{ft}_{gi}")
                            nc.sync.dma_start(
                                hl[:, :cl],
                                hT[ft * 128:(ft + 1) * 128, c0:c0 + cl])
                        for dm in range(8):
                            nc.tensor.matmul(
                                psys[dm][:, :cl],
                                w2ts[ft][:, dm * 128:(dm + 1) * 128],
                                hl[:, :cl],
                                start=(ft == 0), stop=(ft == 31))
                    for dm in range(8):
                        ys = ysp.tile([128, 512], F32, tag="ys",
                                      name=f"ys_{dm}_{gi}")
                        nc.vector.tensor_scalar_add(ys[:, :cl],
                                                    psys[dm][:, :cl],
                                                    b2t[:, dm:dm + 1])
                        nc.vector.tensor_mul(ys[:, :cl], ys[:, :cl],
                                             wbt[:, c0:c0 + cl])
                        nc.sync.dma_start(
                            yT[dm * 128:(dm + 1) * 128, c0:c0 + cl],
                            ys[:, :cl])

            if loop_n == 1:
                body()
            else:
                with tc.For_i(0, loop_n, 1):
                    body()
    nc.compile()
    return nc


def get_progs(C, loop_n=1):
    key = (C, loop_n, MM_MODE)
    if key not in _PROGS:
        _PROGS[key] = build(C, loop_n)
    return _PROGS[key]


def route(x, Wr, br):
    """Host router: top-1 expert index and gate weight (mirrors reference)."""
    logits = x.astype(np.float32) @ Wr.astype(np.float32) + br.astype(np.float32)
    m = logits.max(axis=-1, keepdims=True)
    ex = np.exp(logits - m)
    p = ex / ex.sum(axis=-1, keepdims=True)
    idx = np.argmax(p, axis=-1)
    w = p[np.arange(p.shape[0]), idx]
    return idx, w


# SBUF budget caps the per-core token capacity of one program invocation;
# pathological routing (an expert drawing >C_MAX tokens) falls back to
# running the same program over multiple token chunks.
C_MAX = 4096


def _pack(x, w, sels, C, W1, b1, W2, b2):
    ins = []
    for e in range(E):
        sel = sels[e]
        n = len(sel)
        xTe = np.zeros((D, C), np.float16 if MM_MODE == "fp16" else np.float32)
        xTe[:, :n] = _np_mm(x[sel].T)
        w1h = np.ascontiguousarray(
            _np_mm(W1[e]).reshape(8, 128, 32, 128)
            .transpose(2, 1, 0, 3).reshape(F, D))
        b1h = np.ascontiguousarray(
            np.asarray(b1[e], np.float32).reshape(32, 128).T)
        b2h = np.ascontiguousarray(
            np.asarray(b2[e], np.float32).reshape(8, 128).T)
        wbe = np.zeros((128, C), np.float32)
        wbe[:, :n] = np.asarray(w[sel], np.float32)[None, :]
        ins.append({"xT": xTe, "w1h": w1h,
                    "w2": np.ascontiguousarray(_np_mm(W2[e])),
                    "b1h": b1h, "b2h": b2h, "wb": wbe})
    return ins


def prepare(x, Wr, br, W1, b1, W2, b2):
    """Host routing + per-core input packing (single-chunk case)."""
    x = np.ascontiguousarray(np.asarray(x, np.float32))
    idx, w = route(x, np.asarray(Wr, np.float32), np.asarray(br, np.float32))
    perm = np.argsort(idx, kind="stable")
    counts = np.bincount(idx, minlength=E)
    C = int(max(256, -(-counts.max() // 256) * 256))
    offs = np.zeros(E + 1, np.int64)
    offs[1:] = np.cumsum(counts)
    sels = [perm[offs[e]:offs[e + 1]] for e in range(E)]
    ins = _pack(x, w, sels, min(C, C_MAX), W1, b1, W2, b2)
    return ins, perm, counts, C


def kernel(x, Wr, br, W1, b1, W2, b2):
    x = np.ascontiguousarray(np.asarray(x, np.float32))
    idx, w = route(x, np.asarray(Wr, np.float32), np.asarray(br, np.float32))
    perm = np.argsort(idx, kind="stable")
    counts = np.bincount(idx, minlength=E)
    offs = np.zeros(E + 1, np.int64)
    offs[1:] = np.cumsum(counts)
    sels = [perm[offs[e]:offs[e + 1]] for e in range(E)]
    out = np.zeros((x.shape[0], D), np.float32)
    n_chunks = max(1, -(-int(counts.max()) // C_MAX))
    for ci in range(n_chunks):
        cs = [s[ci * C_MAX:(ci + 1) * C_MAX] for s in sels]
        cmax = max(len(s) for s in cs)
        C = int(max(256, -(-cmax // 256) * 256))
        ins = _pack(x, w, cs, C, W1, b1, W2, b2)
        nc = get_progs(C)
        r = bass_utils.run_bass_kernel_spmd(nc, ins, list(range(N_CORES)))
        for e in range(E):
            n = len(cs[e])
            if n:
                out[cs[e]] = r.results[e]["yT"][:, :n].T
    return out
